# revision 18
# baseline (speedup 1.0000x reference)
"""DistanceLoss kernel for 8 Trainium2 NeuronCores.

Reference computation (T=64, H=32, W=8, B=2048):
    belongs = target.T                              # [T, B] in {0,1}
    iwd  = sum_w inner_window_distances             # [T, H, B]
    cow  = sum_w outer_window_distances             # [T, H, B]
    bl   = belongs*(1-cont)*(ofd + iwd)             # [T, H, B]
    nbl  = (1-belongs)*cont*(ifd + cow)             # [T, H, B]
    loss = mean_b sum_t [ min_h bl + max_h nbl ]

Because c1 = belongs*(1-cont) and c2 = (1-belongs)*cont are constant over h
and take values in {0,1}:  min_h bl == c1 * min_h(ofd+iwd)  and
max_h nbl == c2 * max_h(ifd+cow)  exactly.

Sharding: T is split 8 ways (8 towns per core) so every per-core slab of the
two big [T,H,W,B] tensors is one fully contiguous 16.75 MB region -> maximal
DMA efficiency.  Each core computes a partial [B] loss vector summed over its
8 towns; the host adds the 8 partials and takes the mean.

Hardware constraint honored throughout: a compute instruction can encode at
most ONE sync wait.  DMA'd tiles are consumed only by single-input DVE ops
(copy / reduce) writing into DVE-only pools; tensor_tensor ops touch only
DVE-produced tiles; the PE-visible accumulator is written by a pure-DVE add
whose single wait is the WAR on previous PE readers.

Per-core dataflow (t_loc=8, rows r=(t,h) in 2 partition-tiles of 128):
  1. DMA one (t4,h32) slab of iw:  [128, (w8 b2048)]  (64 KB contig per row)
  2. DVE reduce_add over w (strided AP) -> r;  copy frame -> acc;
     a = acc + r                                  [128(t4,h32), 2048]
  3. PE transpose 4x 128x128 blocks into one PSUM bank [b128, 512]
  4. DVE reduce min (max for ow side) over h   -> m1/m2 [b128, (bc16 t8)]
  5. tiny final combine with c1/c2 from target/containment, reduce over t
  6. DMA out z [128, 16]  (z[p, bc] = partial loss for b = bc*128+p)
"""

import numpy as np

T, H, W, B = 64, 32, 8, 2048
NCORES = 8
TL = T // NCORES          # 8 local towns per core
NBC = B // 128            # 16 batch chunks of 128

_CACHE = {}


def _build_program():
    import concourse.bass as bass
    import concourse.tile as tile
    from concourse import bacc, mybir

    f32 = mybir.dt.float32
    u8 = mybir.dt.uint8
    AX = mybir.AxisListType
    OP = mybir.AluOpType

    nc = bacc.Bacc()
    iw = nc.declare_dram_parameter("iw", [TL, H, W, B], f32, isOutput=False)
    ow = nc.declare_dram_parameter("ow", [TL, H, W, B], f32, isOutput=False)
    ofd = nc.declare_dram_parameter("ofd", [TL, H, B], f32, isOutput=False)
    ifd = nc.declare_dram_parameter("ifd", [TL, H, B], f32, isOutput=False)
    cont = nc.declare_dram_parameter("cont", [TL, B], f32, isOutput=False)
    tgt = nc.declare_dram_parameter("tgt", [B, TL], u8, isOutput=False)
    z = nc.declare_dram_parameter("z", [128, NBC], f32, isOutput=True)

    ident = nc.inline_tensor(np.eye(128, dtype=np.float32), name="ident128")

    with tile.TileContext(nc) as tc:
        with (
            tc.tile_pool(name="const", bufs=1) as const_pool,
            tc.tile_pool(name="big", bufs=3) as big_pool,
            tc.tile_pool(name="frame", bufs=4) as frame_pool,
            tc.tile_pool(name="dve", bufs=2) as dve_pool,
            tc.tile_pool(name="atile", bufs=2) as a_pool,
            tc.tile_pool(name="mres", bufs=1) as m_pool,
            tc.tile_pool(name="fin", bufs=1) as fin_pool,
            tc.tile_pool(name="ps", bufs=4, space="PSUM") as psum_pool,
            tc.tile_pool(name="psc", bufs=1, space="PSUM") as psc_pool,
        ):
            identt = const_pool.tile([128, 128], f32)
            nc.sync.dma_start(identt[:], ident[:, :])
            # route identity through DVE so PE's dependency on it rides the
            # DVE semaphore (merges with data waits; 1-wait limit per inst)
            identc = const_pool.tile([128, 128], f32)
            nc.vector.tensor_copy(identc[:], identt[:])

            # m1/m2: col = bc*TL + t
            m1 = m_pool.tile([128, NBC * TL], f32, tag="m1")
            m2 = m_pool.tile([128, NBC * TL], f32, tag="m2")

            # prefetch all small tensors up front (fresh slots -> waitless
            # DMAs).  Each frame DMA gets a 1-column dummy DVE copy that
            # absorbs the DMA wait; later DVE consumers of the tile are then
            # covered by the vector clock and stay at <=1 wait.
            frs = {}
            for side in range(2):
                src3 = ofd if side == 0 else ifd
                for th in range(2):
                    t0 = th * 4
                    fr = frame_pool.tile([128, B], f32, tag="fr")
                    nc.sync.dma_start(
                        fr[:],
                        src3[t0 : t0 + 4, :, :].rearrange("t h b -> (t h) b"),
                    )
                    dummy = frame_pool.tile([128, 1], f32, tag=f"dum{side}{th}")
                    nc.vector.tensor_copy(dummy[:], fr[:, 0:1])
                    frs[(side, th)] = fr
            tgt8 = fin_pool.tile([128, NBC * TL], u8, tag="tgt8")
            nc.sync.dma_start(
                tgt8[:].rearrange("p (c t) -> p c t", t=TL),
                tgt.rearrange("(c p) t -> p c t", p=128),
            )
            cnat = fin_pool.tile([TL, B], f32, tag="cnat")
            nc.sync.dma_start(cnat[:], cont[:, :])

            for side in range(2):
                src4 = iw if side == 0 else ow
                src3 = ofd if side == 0 else ifd
                mdst = m1 if side == 0 else m2
                red_op = OP.min if side == 0 else OP.max

                for th in range(2):  # rows r = (t_loc in [4*th, 4*th+4), h)
                    t0 = th * 4
                    fr = frs[(side, th)]
                    mview = mdst[:].rearrange("p (c t) -> p c t", t=TL)
                    for bh in range(2):  # b halves of 1024
                        BH = B // 2
                        b0 = bh * BH
                        # (t4,h32) slab, b half: [128, (w8 b1024)], 4 MB,
                        # 32 KB contiguous per partition row
                        bt = big_pool.tile([128, W * BH], f32, tag="big")
                        nc.sync.dma_start(
                            bt[:].rearrange("p (w b) -> p w b", w=W),
                            src4[t0 : t0 + 4, :, :, b0 : b0 + BH].rearrange(
                                "t h w b -> (t h) w b"
                            ),
                        )

                        # r[p, b] = sum_w bt[p, w, b]  (b outer, w inner)
                        r = dve_pool.tile([128, BH], f32, tag="r")
                        nc.vector.tensor_reduce(
                            r[:],
                            bt[:].rearrange("p (w b) -> p b w", w=W),
                            axis=AX.X,
                            op=OP.add,
                        )
                        # a = frame + r  (frame DMA wait already
                        # absorbed by the dummy copy; single WAR-PE wait)
                        a = a_pool.tile([128, BH], f32, tag="a")
                        nc.vector.tensor_add(a[:], fr[:, b0 : b0 + BH], r[:])

                        # 4 b-chunks per PSUM bank.  Same-bank PE writes
                        # chain on the PE self-sem (1 wait each); bufs=4 ->
                        # banks recycle once per half-iteration, so the WAR
                        # on older DVE reduces is covered by the a-add wait
                        # (vector clock) - every instr keeps <=1 wait.
                        for g in range(2):
                            pt = psum_pool.tile([128, 512], f32, tag="pt")
                            for q in range(4):
                                lc = g * 4 + q
                                nc.tensor.transpose(
                                    pt[:, q * 128 : (q + 1) * 128],
                                    a[:, lc * 128 : (lc + 1) * 128],
                                    identc[:],
                                )
                            gg = bh * 2 + g
                            nc.vector.tensor_reduce(
                                mview[:, gg * 4 : (gg + 1) * 4, t0 : t0 + 4],
                                pt[:].rearrange(
                                    "p (c t h) -> p c t h", t=4, h=H
                                ),
                                axis=AX.X,
                                op=red_op,
                            )

            # ---- final combine ----
            bel = fin_pool.tile([128, NBC * TL], f32, tag="bel")
            nc.vector.tensor_copy(bel[:], tgt8[:])

            # containment [t, b] -> [b, (bc t)] via PE transpose (K=8).
            # Route through a DVE copy so the first transpose's single wait
            # is the DVE sem (covers identc as well via the vector clock).
            cnatc = fin_pool.tile([TL, B], f32, tag="cnatc")
            nc.vector.tensor_copy(cnatc[:], cnat[:])
            cT = fin_pool.tile([128, NBC * TL], f32, tag="cT")
            cp = psc_pool.tile([128, NBC * TL], f32, tag="cps")
            for bc in range(NBC):
                nc.tensor.transpose(
                    cp[:, bc * TL : (bc + 1) * TL],
                    cnatc[:, bc * 128 : (bc + 1) * 128],
                    identc[0:TL, 0:TL],
                )
            nc.vector.tensor_copy(cT[:], cp[:])

            # c1 = bel*(1-cT) = bel - bel*cT ; c2 = (1-bel)*cT = cT - bel*cT
            bc_t = fin_pool.tile([128, NBC * TL], f32, tag="bct")
            nc.vector.tensor_mul(bc_t[:], bel[:], cT[:])
            c1 = fin_pool.tile([128, NBC * TL], f32, tag="c1")
            nc.vector.tensor_sub(c1[:], bel[:], bc_t[:])
            c2 = fin_pool.tile([128, NBC * TL], f32, tag="c2")
            nc.vector.tensor_sub(c2[:], cT[:], bc_t[:])

            w1 = fin_pool.tile([128, NBC * TL], f32, tag="w1")
            nc.vector.tensor_mul(w1[:], c1[:], m1[:])
            w2 = fin_pool.tile([128, NBC * TL], f32, tag="w2")
            nc.vector.tensor_mul(w2[:], c2[:], m2[:])
            wt = fin_pool.tile([128, NBC * TL], f32, tag="wt")
            nc.vector.tensor_add(wt[:], w1[:], w2[:])

            zb = fin_pool.tile([128, NBC], f32, tag="zb")
            nc.vector.tensor_reduce(
                zb[:],
                wt[:].rearrange("p (c t) -> p c t", t=TL),
                axis=AX.X,
                op=OP.add,
            )
            nc.sync.dma_start(z[:, :], zb[:])

    nc.finalize()
    return nc


def _get_program():
    if "nc" not in _CACHE:
        _CACHE["nc"] = _build_program()
    return _CACHE["nc"]


def kernel(
    inner_window_distances: np.ndarray,
    outer_window_distances: np.ndarray,
    outer_frame_distance: np.ndarray,
    inner_frame_distance: np.ndarray,
    containment: np.ndarray,
    target: np.ndarray,
) -> np.ndarray:
    from concourse.bass_utils import run_bass_kernel_spmd

    nc = _get_program()

    iw = np.ascontiguousarray(inner_window_distances, dtype=np.float32)
    owd = np.ascontiguousarray(outer_window_distances, dtype=np.float32)
    ofd = np.ascontiguousarray(outer_frame_distance, dtype=np.float32)
    ifd = np.ascontiguousarray(inner_frame_distance, dtype=np.float32)
    cont = np.ascontiguousarray(containment, dtype=np.float32)
    tgt = np.ascontiguousarray(target).view(np.uint8)

    core_ids = list(range(NCORES))
    in_maps = []
    for c in core_ids:
        t0, t1 = c * TL, (c + 1) * TL
        in_maps.append(
            {
                "iw": np.ascontiguousarray(iw[t0:t1]),
                "ow": np.ascontiguousarray(owd[t0:t1]),
                "ofd": np.ascontiguousarray(ofd[t0:t1]),
                "ifd": np.ascontiguousarray(ifd[t0:t1]),
                "cont": np.ascontiguousarray(cont[t0:t1]),
                "tgt": np.ascontiguousarray(tgt[:, t0:t1]),
            }
        )

    res = run_bass_kernel_spmd(nc, in_maps, core_ids)

    # z[p, bc] (per core) = partial loss for b = bc*128 + p, summed over the
    # core's 8 towns.  Sum cores, flatten to [B], mean.
    acc = np.zeros((128, NBC), dtype=np.float64)
    for r in res.results:
        acc += r["z"].astype(np.float64)
    loss_b = acc.T.reshape(B)
    return np.float32(loss_b.mean())
